# revision 7
# baseline (speedup 1.0000x reference)
"""DendriticMLP Trainium2 kernel — 8-core batch-data-parallel, fp16 3-pass.

Per core (B_local=512 rows):
  All matmuls run as fp16 hi/lo 3-pass splits (hh + lh + hl) accumulated
  in one fp32 PSUM chain: measured on HW this matches native fp32 to
  ~2e-7 rel while streaming at 1 cycle/row (vs 4 for fp32).

  y1 = x @ w1.T + b1                (PE)
  dend1 = ctx @ seg1_flat.T         (PE) -> per-(b,h) max/min over 10
          segments (DVE strided reduce from PSUM) -> sel =
          where(max+min>0, max, min) -> gate = sigmoid(sel) (ACT)
  g = y1 * gate; top-k (k=102) per row via threshold bisection on
          count(g >= t), 23 iterations; h = (g >= lo) * g
  h transposed 128x128 blocks on PE (fp32), split to fp16 hi/lo while
  copying out of PSUM; layer 2 same; out = h2 @ w_out.T + b_out.
"""
import numpy as np
from contextlib import ExitStack

import concourse.bass as bass
import concourse.tile as tile
from concourse import bacc, mybir, masks
from concourse.bass_utils import run_bass_kernel_spmd

F32 = mybir.dt.float32
F16 = mybir.dt.float16
AF = mybir.ActivationFunctionType
OP = mybir.AluOpType
AX = mybir.AxisListType

# problem dims (hardcoded per contract)
B, D_IN, H, S, D_CTX, D_OUT = 4096, 1024, 2048, 10, 1024, 1024
KW = 102                 # k-winners per row
NCORES = 8
BL = B // NCORES         # 512 rows per core
BT = BL // 128           # 4 b-tiles of 128 rows
NITER = 23               # bisection iterations

HS = H * S               # 20480
CHW = 510                # dend chunk width (51 groups of 10)
NCH = HS // CHW          # 40 full chunks
TAIL = HS - NCH * CHW    # 80 (8 groups)
GR = CHW // S            # 51 groups per chunk
KT_IN = D_IN // 128      # 8 contraction tiles for d=1024
KT_H = H // 128          # 16 contraction tiles for d=2048


def build_kernel(loop_n=None):
    nc = bacc.Bacc("TRN2", target_bir_lowering=False, debug=False,
                   num_devices=NCORES)

    def din(name, shape, dt=F16):
        return nc.dram_tensor(name, shape, dt, kind="ExternalInput").ap()

    xT_h = din("xT_h", [D_IN, BL])
    xT_l = din("xT_l", [D_IN, BL])
    ctxT_h = din("ctxT_h", [D_IN, BL])
    ctxT_l = din("ctxT_l", [D_IN, BL])
    w1t_h = din("w1t_h", [H // 512, KT_IN, 128, 512])
    w1t_l = din("w1t_l", [H // 512, KT_IN, 128, 512])
    w2t_h = din("w2t_h", [H // 512, KT_H, 128, 512])
    w2t_l = din("w2t_l", [H // 512, KT_H, 128, 512])
    wot_h = din("wot_h", [D_OUT // 512, KT_H, 128, 512])
    wot_l = din("wot_l", [D_OUT // 512, KT_H, 128, 512])
    sg1a_h = din("sg1a_h", [NCH, KT_IN, 128, CHW])
    sg1a_l = din("sg1a_l", [NCH, KT_IN, 128, CHW])
    sg1b_h = din("sg1b_h", [KT_IN, 128, TAIL])
    sg1b_l = din("sg1b_l", [KT_IN, 128, TAIL])
    sg2a_h = din("sg2a_h", [NCH, KT_IN, 128, CHW])
    sg2a_l = din("sg2a_l", [NCH, KT_IN, 128, CHW])
    sg2b_h = din("sg2b_h", [KT_IN, 128, TAIL])
    sg2b_l = din("sg2b_l", [KT_IN, 128, TAIL])
    b1d = din("b1d", [1, H], F32)
    b2d = din("b2d", [1, H], F32)
    bod = din("bod", [1, D_OUT], F32)
    out_d = nc.dram_tensor("out", [BL, D_OUT], F32, kind="ExternalOutput").ap()

    with tile.TileContext(nc) as tc, ExitStack() as ctx:
        if loop_n is not None:
            ctx.enter_context(tc.For_i(0, loop_n, 1))
        cpool = ctx.enter_context(tc.tile_pool(name="const", bufs=1))
        apool = ctx.enter_context(tc.tile_pool(name="acts", bufs=1))
        ypool = ctx.enter_context(tc.tile_pool(name="y", bufs=BT))
        selpool = ctx.enter_context(tc.tile_pool(name="sel", bufs=BT))
        mnpool = ctx.enter_context(tc.tile_pool(name="mn", bufs=BT))
        htpool = ctx.enter_context(tc.tile_pool(name="ht", bufs=1))
        wpool = ctx.enter_context(tc.tile_pool(name="w", bufs=4))
        spool = ctx.enter_context(tc.tile_pool(name="seg", bufs=16))
        outpool = ctx.enter_context(tc.tile_pool(name="osb", bufs=2))
        tinypool = ctx.enter_context(tc.tile_pool(name="tiny", bufs=1))
        biaspool = ctx.enter_context(tc.tile_pool(name="bias", bufs=1))
        psy = ctx.enter_context(tc.tile_pool(name="psy", bufs=BT, space="PSUM"))
        psd = ctx.enter_context(tc.tile_pool(name="psd", bufs=2, space="PSUM"))
        pst = ctx.enter_context(tc.tile_pool(name="pst", bufs=2, space="PSUM"))

        # constants
        identity = cpool.tile([128, 128], F32)
        masks.make_identity(nc, identity[:])
        ones = cpool.tile([1, 128], F32)
        nc.gpsimd.memset(ones[:], 1.0)

        def load_bias(src, width):
            t = biaspool.tile([1, H], F32, tag="bias", name="bias_sb")
            nc.sync.dma_start(t[:, :width], src)
            return t

        b1sb = load_bias(b1d, H)

        # activations stationary: [128, kk*BL + bt*128] layout, hi/lo fp16
        xT_sb_h = apool.tile([128, KT_IN * BL], F16, tag="xth")
        xT_sb_l = apool.tile([128, KT_IN * BL], F16)
        ctxT_sb_h = apool.tile([128, KT_IN * BL], F16)
        ctxT_sb_l = apool.tile([128, KT_IN * BL], F16)
        for kk in range(KT_IN):
            sl = slice(kk * BL, (kk + 1) * BL)
            rows = slice(kk * 128, (kk + 1) * 128)
            nc.sync.dma_start(xT_sb_h[:, sl], xT_h[rows, :])
            nc.sync.dma_start(xT_sb_l[:, sl], xT_l[rows, :])
            nc.sync.dma_start(ctxT_sb_h[:, sl], ctxT_h[rows, :])
            nc.sync.dma_start(ctxT_sb_l[:, sl], ctxT_l[rows, :])

        def st_ap(sb, kk, bt):
            return sb[:, kk * BL + bt * 128: kk * BL + (bt + 1) * 128]

        # per-bt tiny state: cols 0=M 1=lo 2=w 3=t 4=pred 5=cnt
        tiny = [tinypool.tile([128, 8], F32, tag=f"tiny{bt}", name=f"tiny{bt}")
                for bt in range(BT)]

        # h transposed, fp16 hi/lo (reused for h2T)
        h1T_h = htpool.tile([128, KT_H * 512], F16)
        h1T_l = htpool.tile([128, KT_H * 512], F16)
        scr_cell = []

        def get_scr():
            if not scr_cell:
                # reuses xT_sb_h's buffer (dead after layer-1 y matmuls)
                scr_cell.append(apool.tile([128, H], F32, tag="xth",
                                           name="scr"))
            return scr_cell[0]

        def yphase(stat_h, stat_l, lay, kt, wt_h, wt_l, bias_sb, width):
            """Dense y = act @ W.T (+bias), fp16 3-pass.
            Returns list of BT y tiles [128, width] fp32."""
            ytiles = [ypool.tile([128, H], F32, tag="y", name=f"y{lay}_{i}")
                      for i in range(BT)]
            nch = width // 512
            for n in range(nch):
                ps = [psy.tile([128, 512], F32, tag="psy", name=f"psy{i}")
                      for i in range(BT)]
                for k in range(kt):
                    wh = wpool.tile([128, 512], F16, tag="w", name="wh")
                    nc.sync.dma_start(wh[:], wt_h[n, k])
                    wl = wpool.tile([128, 512], F16, tag="w", name="wl")
                    nc.sync.dma_start(wl[:], wt_l[n, k])
                    for bt in range(BT):
                        if lay == 0:
                            sh = st_ap(stat_h, k, bt)
                            sl_ = st_ap(stat_l, k, bt)
                        else:
                            cs = slice(k * 512 + bt * 128,
                                       k * 512 + (bt + 1) * 128)
                            sh = stat_h[:, cs]
                            sl_ = stat_l[:, cs]
                        nc.tensor.matmul(ps[bt][:], sh, wh[:],
                                         start=(k == 0), stop=False)
                        nc.tensor.matmul(ps[bt][:], sl_, wh[:],
                                         start=False, stop=False)
                        nc.tensor.matmul(ps[bt][:], sh, wl[:],
                                         start=False, stop=False)
                for bt in range(BT):
                    nc.tensor.matmul(ps[bt][:], ones[:],
                                     bias_sb[:, n * 512:(n + 1) * 512],
                                     start=False, stop=True)
                for bt in range(BT):
                    nc.scalar.activation(ytiles[bt][:, n * 512:(n + 1) * 512],
                                         ps[bt][:], AF.Copy)
            return ytiles

        def dendphase(sga_h, sga_l, sgb_h, sgb_l, seltiles, mntiles,
                      post_chunk=None):
            """dend matmuls (fp16 3-pass) + segment max/min reduces."""
            for c in range(NCH + 1):
                w = CHW if c < NCH else TAIL
                segs_h, segs_l = [], []
                for k in range(KT_IN):
                    sh = spool.tile([128, CHW], F16, tag="seg", name="sgh")
                    sl_ = spool.tile([128, CHW], F16, tag="seg", name="sgl")
                    if c < NCH:
                        nc.sync.dma_start(sh[:, :w], sga_h[c, k])
                        nc.sync.dma_start(sl_[:, :w], sga_l[c, k])
                    else:
                        nc.sync.dma_start(sh[:, :w], sgb_h[k])
                        nc.sync.dma_start(sl_[:, :w], sgb_l[k])
                    segs_h.append(sh)
                    segs_l.append(sl_)
                g = GR if c < NCH else TAIL // S
                for bt in range(BT):
                    pd = psd.tile([128, CHW], F32, tag="psd")
                    for k in range(KT_IN):
                        nc.tensor.matmul(pd[:, :w], st_ap(ctxT_sb_h, k, bt),
                                         segs_h[k][:, :w],
                                         start=(k == 0), stop=False)
                    for k in range(KT_IN):
                        nc.tensor.matmul(pd[:, :w], st_ap(ctxT_sb_l, k, bt),
                                         segs_h[k][:, :w],
                                         start=False, stop=False)
                    for k in range(KT_IN):
                        nc.tensor.matmul(pd[:, :w], st_ap(ctxT_sb_h, k, bt),
                                         segs_l[k][:, :w],
                                         start=False, stop=(k == KT_IN - 1))
                    view = pd[:, :w].rearrange("p (g s) -> p g s", s=S)
                    nc.vector.tensor_reduce(
                        seltiles[bt][:, c * GR:c * GR + g], view,
                        axis=AX.X, op=OP.max)
                    nc.vector.tensor_reduce(
                        mntiles[bt][:, c * GR:c * GR + g], view,
                        axis=AX.X, op=OP.min)
                if post_chunk is not None:
                    post_chunk(c)

        def selgate(seltiles, mntiles):
            """sel=where(mx+mn>0,mx,mn) in-place over mx, then sigmoid."""
            for bt in range(BT):
                mx = seltiles[bt][:]
                mn = mntiles[bt][:]
                scr = get_scr()
                nc.vector.tensor_add(scr[:], mx, mn)
                nc.vector.tensor_scalar(scr[:], scr[:], 0.0, None, op0=OP.is_le)
                nc.vector.copy_predicated(mx, scr[:].bitcast(mybir.dt.int32), mn)
                nc.scalar.activation(mx, mx, AF.Sigmoid)

        def gate_mul(ytiles, seltiles):
            """g = y*gate in place on y tile; M = absmax(g) into tiny col 0."""
            for bt in range(BT):
                nc.vector.tensor_mul(ytiles[bt][:], ytiles[bt][:],
                                     seltiles[bt][:])
                nc.vector.tensor_reduce(tiny[bt][:, 0:1], ytiles[bt][:],
                                        axis=AX.X, op=OP.max,
                                        apply_absolute_value=True)

        def bisect_init(bt):
            t = tiny[bt]
            M, lo, w, tt_ = (t[:, i:i + 1] for i in range(4))
            nc.vector.tensor_scalar(tt_, M, 1.001, 1e-30, op0=OP.mult,
                                    op1=OP.add)
            nc.vector.tensor_scalar_mul(lo, tt_, -1.0)
            nc.vector.tensor_scalar_mul(w, tt_, 2.0)

        def bisect_iter(ytile, bt, act_scr=None):
            t = tiny[bt]
            M, lo, w, tt_, pred, cnt = (t[:, i:i + 1] for i in range(6))
            tneg = t[:, 6:7]
            nc.vector.tensor_scalar_mul(w, w, 0.5)
            if act_scr is not None:
                # s = sum(sign(g - t)); count>=KW  <=>  s >= 2*KW - H
                nc.vector.scalar_tensor_tensor(tneg, lo, -1.0, w,
                                               op0=OP.mult,
                                               op1=OP.subtract)
                nc.scalar.activation(act_scr[:], ytile[:], AF.Sign,
                                     bias=tneg, accum_out=cnt)
                nc.vector.tensor_scalar(pred, cnt, float(2 * KW - H),
                                        None, op0=OP.is_ge)
            else:
                nc.vector.tensor_add(tt_, lo, w)
                nc.vector.scalar_tensor_tensor(
                    get_scr()[:], ytile[:], tt_, ytile[:],
                    op0=OP.is_ge, op1=OP.bypass, accum_out=cnt)
                nc.vector.tensor_scalar(pred, cnt, float(KW), None,
                                        op0=OP.is_ge)
            nc.vector.scalar_tensor_tensor(lo, pred, w, lo,
                                           op0=OP.mult, op1=OP.add)

        def bisect_fin(ytile, bt):
            lo = tiny[bt][:, 1:2]
            nc.vector.scalar_tensor_tensor(ytile[:], ytile[:], lo, ytile[:],
                                           op0=OP.is_ge, op1=OP.mult)

        def bisect_chain(ytile, bt, act_scr=None):
            """One b-tile's top-k bisection + in-place mask.
            act_scr: if given, counts run on ACT via Sign+accum."""
            bisect_init(bt)
            for it in range(NITER):
                bisect_iter(ytile, bt, act_scr)
            bisect_fin(ytile, bt)

        def transpose_bt(ytile, bt, dst_h, dst_l):
            """PE-transpose fp32 y tile; split hi/lo fp16 on the way out."""
            for kb in range(KT_H):
                pt = pst.tile([128, 128], F32, tag="pst", name="ptb")
                nc.tensor.transpose(pt[:],
                                    ytile[:, kb * 128:(kb + 1) * 128],
                                    identity[:])
                cs = slice(kb * 512 + bt * 128, kb * 512 + (bt + 1) * 128)
                nc.scalar.activation(dst_h[:, cs], pt[:], AF.Copy)
                nc.vector.tensor_sub(dst_l[:, cs], pt[:], dst_h[:, cs])

        def transpose_to(ytiles, dst_h, dst_l):
            for bt in range(BT):
                transpose_bt(ytiles[bt], bt, dst_h, dst_l)

        # ---------------- layer 1 ----------------
        sel1 = [selpool.tile([128, H], F32, tag="sel", name=f"sel1_{i}")
                for i in range(BT)]
        mn1 = [mnpool.tile([128, H], F32, tag="mn", name=f"mn1_{i}")
               for i in range(BT)]
        y1 = yphase(xT_sb_h, xT_sb_l, 0, KT_IN, w1t_h, w1t_l, b1sb, H)
        dendphase(sg1a_h, sg1a_l, sg1b_h, sg1b_l, sel1, mn1)
        selgate(sel1, mn1)
        gate_mul(y1, sel1)
        for bt in range(BT):
            bisect_init(bt)

        # layer-1 bisection emitted interleaved with dend2's chunks so the
        # FIFO DVE queue services dend2's PSUM reduces promptly (otherwise
        # the 2 psd banks fill and the PE stalls behind the bisect chains).
        steps = [(bt, j) for j in range(NITER + 1) for bt in range(BT)]
        step_pos = [0]

        def emit_bisect_steps(n):
            for _ in range(n):
                if step_pos[0] >= len(steps):
                    return
                bt, j = steps[step_pos[0]]
                step_pos[0] += 1
                if j < NITER:
                    bisect_iter(y1[bt], bt)
                else:
                    bisect_fin(y1[bt], bt)

        # dend2 early (keeps PE busy during layer-1 bisection)
        sel2 = [selpool.tile([128, H], F32, tag="sel", name=f"sel2_{i}")
                for i in range(BT)]
        mn2 = [mnpool.tile([128, H], F32, tag="mn", name=f"mn2_{i}")
               for i in range(BT)]
        dendphase(sg2a_h, sg2a_l, sg2b_h, sg2b_l, sel2, mn2,
                  post_chunk=lambda c: emit_bisect_steps(3))
        emit_bisect_steps(len(steps))
        selgate(sel2, mn2)

        transpose_to(y1, h1T_h, h1T_l)

        # ---------------- layer 2 ----------------
        b2sb = load_bias(b2d, H)
        y2 = yphase(h1T_h, h1T_l, 1, KT_H, w2t_h, w2t_l, b2sb, H)
        gate_mul(y2, sel2)

        # per-bt tail pipeline: bisect (DVE/ACT split) -> transpose -> out,
        # so bt0's output matmuls overlap bt1..3's bisection chains.
        bosb = load_bias(bod, D_OUT)
        act_scr = mnpool.tile([128, H], F32, tag="mn", name="act_scr")
        for bt in range(BT):
            bisect_chain(y2[bt], bt, act_scr=(act_scr if bt % 2 else None))
            transpose_bt(y2[bt], bt, h1T_h, h1T_l)
            for n in range(D_OUT // 512):
                ps1 = psy.tile([128, 512], F32, tag="psy", name="pso")
                for k in range(KT_H):
                    wh = wpool.tile([128, 512], F16, tag="w", name="owh")
                    nc.sync.dma_start(wh[:], wot_h[n, k])
                    wl = wpool.tile([128, 512], F16, tag="w", name="owl")
                    nc.sync.dma_start(wl[:], wot_l[n, k])
                    cs = slice(k * 512 + bt * 128, k * 512 + (bt + 1) * 128)
                    nc.tensor.matmul(ps1[:], h1T_h[:, cs], wh[:],
                                     start=(k == 0), stop=False)
                    nc.tensor.matmul(ps1[:], h1T_l[:, cs], wh[:],
                                     start=False, stop=False)
                    nc.tensor.matmul(ps1[:], h1T_h[:, cs], wl[:],
                                     start=False, stop=False)
                nc.tensor.matmul(ps1[:], ones[:],
                                 bosb[:, n * 512:(n + 1) * 512],
                                 start=False, stop=True)
                osb = outpool.tile([128, 512], F32, tag="osb")
                nc.scalar.activation(osb[:], ps1[:], AF.Copy)
                nc.sync.dma_start(
                    out_d[bt * 128:(bt + 1) * 128, n * 512:(n + 1) * 512],
                    osb[:])

    nc.compile()
    return nc


def _split16(a):
    a = np.asarray(a, np.float32)
    hi = a.astype(np.float16)
    lo = (a - hi.astype(np.float32)).astype(np.float16)
    return hi, lo


def _prep_inputs(x, context, w1, b1, seg1, w2, b2, seg2, w_out, b_out):
    """Host-side reshapes into the DMA-friendly tiled layouts + fp16 split."""
    c = np.ascontiguousarray

    def tile_wt(w, kt, nch):
        # w [out, in] -> wT [in, out] -> [nch, kt, 128, 512]
        wT = np.asarray(w, np.float32).T
        return c(wT.reshape(kt, 128, nch, 512).transpose(2, 0, 1, 3))

    def tile_seg(seg):
        segT = np.asarray(seg, np.float32).reshape(HS, D_CTX).T  # [D_CTX, HS]
        a = c(segT[:, :NCH * CHW].reshape(KT_IN, 128, NCH, CHW)
              .transpose(2, 0, 1, 3))
        b = c(segT[:, NCH * CHW:].reshape(KT_IN, 128, TAIL))
        return a, b

    shared = {}
    for name, arr in (("w1t", tile_wt(w1, KT_IN, H // 512)),
                      ("w2t", tile_wt(w2, KT_H, H // 512)),
                      ("wot", tile_wt(w_out, KT_H, D_OUT // 512))):
        hi, lo = _split16(arr)
        shared[name + "_h"] = hi
        shared[name + "_l"] = lo
    for name, seg in (("sg1", seg1), ("sg2", seg2)):
        a, b = tile_seg(seg)
        for suf, arr in (("a", a), ("b", b)):
            hi, lo = _split16(arr)
            shared[name + suf + "_h"] = hi
            shared[name + suf + "_l"] = lo
    shared["b1d"] = c(np.asarray(b1, np.float32).reshape(1, H))
    shared["b2d"] = c(np.asarray(b2, np.float32).reshape(1, H))
    shared["bod"] = c(np.asarray(b_out, np.float32).reshape(1, D_OUT))

    x = np.asarray(x, np.float32)
    context = np.asarray(context, np.float32)
    in_maps = []
    for core in range(NCORES):
        sl = slice(core * BL, (core + 1) * BL)
        m = dict(shared)
        for name, arr in (("xT", c(x[sl].T)), ("ctxT", c(context[sl].T))):
            hi, lo = _split16(arr)
            m[name + "_h"] = hi
            m[name + "_l"] = lo
        in_maps.append(m)
    return in_maps


_NC = None


def kernel(**inputs):
    global _NC
    if _NC is None:
        _NC = build_kernel()
    inputs = {k: np.ascontiguousarray(np.asarray(v), dtype=np.float32)
              for k, v in inputs.items()}
    in_maps = _prep_inputs(**inputs)
    res = run_bass_kernel_spmd(_NC, in_maps, list(range(NCORES)))
    return np.concatenate([res.results[i]["out"] for i in range(NCORES)],
                          axis=0)


# revision 9
# speedup vs baseline: 1.0858x; 1.0858x over previous
"""DendriticMLP Trainium2 kernel — 8-core batch-data-parallel, fp16 3-pass.

Per core (B_local=512 rows):
  All matmuls run as fp16 hi/lo 3-pass splits (hh + lh + hl) accumulated
  in one fp32 PSUM chain: measured on HW this matches native fp32 to
  ~2e-7 rel while streaming at 1 cycle/row (vs 4 for fp32).

  y1 = x @ w1.T + b1                (PE)
  dend1 = ctx @ seg1_flat.T         (PE) -> per-(b,h) max/min over 10
          segments (DVE strided reduce from PSUM) -> sel =
          where(max+min>0, max, min) -> gate = sigmoid(sel) (ACT)
  g = y1 * gate; top-k (k=102) per row via threshold bisection on
          count(g >= t), 23 iterations; h = (g >= lo) * g
  h transposed 128x128 blocks on PE (fp32), split to fp16 hi/lo while
  copying out of PSUM; layer 2 same; out = h2 @ w_out.T + b_out.
"""
import numpy as np
from contextlib import ExitStack

import concourse.bass as bass
import concourse.tile as tile
from concourse import bacc, mybir, masks
from concourse.bass_utils import run_bass_kernel_spmd

F32 = mybir.dt.float32
F16 = mybir.dt.float16
AF = mybir.ActivationFunctionType
OP = mybir.AluOpType
AX = mybir.AxisListType

# problem dims (hardcoded per contract)
B, D_IN, H, S, D_CTX, D_OUT = 4096, 1024, 2048, 10, 1024, 1024
KW = 102                 # k-winners per row
NCORES = 8
BL = B // NCORES         # 512 rows per core
BT = BL // 128           # 4 b-tiles of 128 rows
NITER = 23               # bisection iterations

HS = H * S               # 20480
CHW = 510                # dend chunk width (51 groups of 10)
NCH = HS // CHW          # 40 full chunks
TAIL = HS - NCH * CHW    # 80 (8 groups)
GR = CHW // S            # 51 groups per chunk
KT_IN = D_IN // 128      # 8 contraction tiles for d=1024
KT_H = H // 128          # 16 contraction tiles for d=2048

# emit layer-1 bisection interleaved with dend2 chunks (FIFO-DVE fairness)
INTERLEAVE = False


def build_kernel(loop_n=None):
    nc = bacc.Bacc("TRN2", target_bir_lowering=False, debug=False,
                   num_devices=NCORES)

    def din(name, shape, dt=F16):
        return nc.dram_tensor(name, shape, dt, kind="ExternalInput").ap()

    xT_h = din("xT_h", [D_IN, BL])
    xT_l = din("xT_l", [D_IN, BL])
    ctxT_h = din("ctxT_h", [D_IN, BL])
    ctxT_l = din("ctxT_l", [D_IN, BL])
    w1t_h = din("w1t_h", [H // 512, KT_IN, 128, 512])
    w1t_l = din("w1t_l", [H // 512, KT_IN, 128, 512])
    w2t_h = din("w2t_h", [H // 512, KT_H, 128, 512])
    w2t_l = din("w2t_l", [H // 512, KT_H, 128, 512])
    wot_h = din("wot_h", [D_OUT // 512, KT_H, 128, 512])
    wot_l = din("wot_l", [D_OUT // 512, KT_H, 128, 512])
    sg1a_h = din("sg1a_h", [NCH, KT_IN, 128, CHW])
    sg1a_l = din("sg1a_l", [NCH, KT_IN, 128, CHW])
    sg1b_h = din("sg1b_h", [KT_IN, 128, TAIL])
    sg1b_l = din("sg1b_l", [KT_IN, 128, TAIL])
    sg2a_h = din("sg2a_h", [NCH, KT_IN, 128, CHW])
    sg2a_l = din("sg2a_l", [NCH, KT_IN, 128, CHW])
    sg2b_h = din("sg2b_h", [KT_IN, 128, TAIL])
    sg2b_l = din("sg2b_l", [KT_IN, 128, TAIL])
    b1d = din("b1d", [1, H], F32)
    b2d = din("b2d", [1, H], F32)
    bod = din("bod", [1, D_OUT], F32)
    out_d = nc.dram_tensor("out", [BL, D_OUT], F32, kind="ExternalOutput").ap()

    with tile.TileContext(nc) as tc, ExitStack() as ctx:
        if loop_n is not None:
            ctx.enter_context(tc.For_i(0, loop_n, 1))
        cpool = ctx.enter_context(tc.tile_pool(name="const", bufs=1))
        apool = ctx.enter_context(tc.tile_pool(name="acts", bufs=1))
        ypool = ctx.enter_context(tc.tile_pool(name="y", bufs=BT))
        selpool = ctx.enter_context(tc.tile_pool(name="sel", bufs=BT))
        mnpool = ctx.enter_context(tc.tile_pool(name="mn", bufs=BT))
        htpool = ctx.enter_context(tc.tile_pool(name="ht", bufs=1))
        wpool = ctx.enter_context(tc.tile_pool(name="w", bufs=4))
        spool = ctx.enter_context(tc.tile_pool(name="seg", bufs=16))
        outpool = ctx.enter_context(tc.tile_pool(name="osb", bufs=2))
        tinypool = ctx.enter_context(tc.tile_pool(name="tiny", bufs=1))
        biaspool = ctx.enter_context(tc.tile_pool(name="bias", bufs=1))
        psy = ctx.enter_context(tc.tile_pool(name="psy", bufs=BT, space="PSUM"))
        psd = ctx.enter_context(tc.tile_pool(name="psd", bufs=2, space="PSUM"))
        pst = ctx.enter_context(tc.tile_pool(name="pst", bufs=2, space="PSUM"))

        # constants
        identity = cpool.tile([128, 128], F32)
        masks.make_identity(nc, identity[:])
        ones = cpool.tile([1, 128], F32)
        nc.gpsimd.memset(ones[:], 1.0)

        def load_bias(src, width):
            t = biaspool.tile([1, H], F32, tag="bias", name="bias_sb")
            nc.sync.dma_start(t[:, :width], src)
            return t

        b1sb = load_bias(b1d, H)

        # activations stationary: [128, kk*BL + bt*128] layout, hi/lo fp16
        xT_sb_h = apool.tile([128, KT_IN * BL], F16, tag="xth")
        xT_sb_l = apool.tile([128, KT_IN * BL], F16)
        ctxT_sb_h = apool.tile([128, KT_IN * BL], F16)
        ctxT_sb_l = apool.tile([128, KT_IN * BL], F16)
        for kk in range(KT_IN):
            sl = slice(kk * BL, (kk + 1) * BL)
            rows = slice(kk * 128, (kk + 1) * 128)
            nc.sync.dma_start(xT_sb_h[:, sl], xT_h[rows, :])
            nc.sync.dma_start(xT_sb_l[:, sl], xT_l[rows, :])
            nc.sync.dma_start(ctxT_sb_h[:, sl], ctxT_h[rows, :])
            nc.sync.dma_start(ctxT_sb_l[:, sl], ctxT_l[rows, :])

        def st_ap(sb, kk, bt):
            return sb[:, kk * BL + bt * 128: kk * BL + (bt + 1) * 128]

        # per-bt tiny state: cols 0=M 1=lo 2=w 3=t 4=pred 5=cnt
        tiny = [tinypool.tile([128, 8], F32, tag=f"tiny{bt}", name=f"tiny{bt}")
                for bt in range(BT)]

        # h transposed, fp16 hi/lo (reused for h2T)
        h1T_h = htpool.tile([128, KT_H * 512], F16)
        h1T_l = htpool.tile([128, KT_H * 512], F16)
        scr_cell = []

        def get_scr():
            if not scr_cell:
                # reuses xT_sb_h's buffer (dead after layer-1 y matmuls)
                scr_cell.append(apool.tile([128, H], F32, tag="xth",
                                           name="scr"))
            return scr_cell[0]

        def yphase(stat_h, stat_l, lay, kt, wt_h, wt_l, bias_sb, width):
            """Dense y = act @ W.T (+bias), fp16 3-pass.
            Returns list of BT y tiles [128, width] fp32."""
            ytiles = [ypool.tile([128, H], F32, tag="y", name=f"y{lay}_{i}")
                      for i in range(BT)]
            nch = width // 512
            for n in range(nch):
                ps = [psy.tile([128, 512], F32, tag="psy", name=f"psy{i}")
                      for i in range(BT)]
                for k in range(kt):
                    wh = wpool.tile([128, 512], F16, tag="w", name="wh")
                    nc.sync.dma_start(wh[:], wt_h[n, k])
                    wl = wpool.tile([128, 512], F16, tag="w", name="wl")
                    nc.sync.dma_start(wl[:], wt_l[n, k])
                    for bt in range(BT):
                        if lay == 0:
                            sh = st_ap(stat_h, k, bt)
                            sl_ = st_ap(stat_l, k, bt)
                        else:
                            cs = slice(k * 512 + bt * 128,
                                       k * 512 + (bt + 1) * 128)
                            sh = stat_h[:, cs]
                            sl_ = stat_l[:, cs]
                        nc.tensor.matmul(ps[bt][:], sh, wh[:],
                                         start=(k == 0), stop=False)
                        nc.tensor.matmul(ps[bt][:], sl_, wh[:],
                                         start=False, stop=False)
                        nc.tensor.matmul(ps[bt][:], sh, wl[:],
                                         start=False, stop=False)
                for bt in range(BT):
                    nc.tensor.matmul(ps[bt][:], ones[:],
                                     bias_sb[:, n * 512:(n + 1) * 512],
                                     start=False, stop=True)
                for bt in range(BT):
                    nc.scalar.activation(ytiles[bt][:, n * 512:(n + 1) * 512],
                                         ps[bt][:], AF.Copy)
            return ytiles

        def dendphase(sga_h, sga_l, sgb_h, sgb_l, seltiles, mntiles,
                      post_chunk=None):
            """dend matmuls (fp16 3-pass) + segment max/min reduces."""
            for c in range(NCH + 1):
                w = CHW if c < NCH else TAIL
                segs_h, segs_l = [], []
                for k in range(KT_IN):
                    sh = spool.tile([128, CHW], F16, tag="seg", name="sgh")
                    sl_ = spool.tile([128, CHW], F16, tag="seg", name="sgl")
                    if c < NCH:
                        nc.sync.dma_start(sh[:, :w], sga_h[c, k])
                        nc.sync.dma_start(sl_[:, :w], sga_l[c, k])
                    else:
                        nc.sync.dma_start(sh[:, :w], sgb_h[k])
                        nc.sync.dma_start(sl_[:, :w], sgb_l[k])
                    segs_h.append(sh)
                    segs_l.append(sl_)
                g = GR if c < NCH else TAIL // S
                for bt in range(BT):
                    pd = psd.tile([128, CHW], F32, tag="psd")
                    for k in range(KT_IN):
                        nc.tensor.matmul(pd[:, :w], st_ap(ctxT_sb_h, k, bt),
                                         segs_h[k][:, :w],
                                         start=(k == 0), stop=False)
                    for k in range(KT_IN):
                        nc.tensor.matmul(pd[:, :w], st_ap(ctxT_sb_l, k, bt),
                                         segs_h[k][:, :w],
                                         start=False, stop=False)
                    for k in range(KT_IN):
                        nc.tensor.matmul(pd[:, :w], st_ap(ctxT_sb_h, k, bt),
                                         segs_l[k][:, :w],
                                         start=False, stop=(k == KT_IN - 1))
                    view = pd[:, :w].rearrange("p (g s) -> p g s", s=S)
                    nc.vector.tensor_reduce(
                        seltiles[bt][:, c * GR:c * GR + g], view,
                        axis=AX.X, op=OP.max)
                    nc.vector.tensor_reduce(
                        mntiles[bt][:, c * GR:c * GR + g], view,
                        axis=AX.X, op=OP.min)
                if post_chunk is not None:
                    post_chunk(c)

        def selgate(seltiles, mntiles):
            """sel=where(mx+mn>0,mx,mn) in-place over mx, then sigmoid."""
            for bt in range(BT):
                mx = seltiles[bt][:]
                mn = mntiles[bt][:]
                scr = get_scr()
                nc.vector.tensor_add(scr[:], mx, mn)
                nc.vector.tensor_scalar(scr[:], scr[:], 0.0, None, op0=OP.is_le)
                nc.vector.copy_predicated(mx, scr[:].bitcast(mybir.dt.int32), mn)
                nc.scalar.activation(mx, mx, AF.Sigmoid)

        def gate_mul(ytiles, seltiles):
            """g = y*gate in place on y tile; M = absmax(g) into tiny col 0."""
            for bt in range(BT):
                nc.vector.tensor_mul(ytiles[bt][:], ytiles[bt][:],
                                     seltiles[bt][:])
                nc.vector.tensor_reduce(tiny[bt][:, 0:1], ytiles[bt][:],
                                        axis=AX.X, op=OP.max,
                                        apply_absolute_value=True)

        def bisect_init(bt):
            t = tiny[bt]
            M, lo, w, tt_ = (t[:, i:i + 1] for i in range(4))
            nc.vector.tensor_scalar(tt_, M, 1.001, 1e-30, op0=OP.mult,
                                    op1=OP.add)
            nc.vector.tensor_scalar_mul(lo, tt_, -1.0)
            nc.vector.tensor_scalar_mul(w, tt_, 2.0)

        def bisect_iter(ytile, bt, act_scr=None):
            t = tiny[bt]
            M, lo, w, tt_, pred, cnt = (t[:, i:i + 1] for i in range(6))
            tneg = t[:, 6:7]
            nc.vector.tensor_scalar_mul(w, w, 0.5)
            if act_scr is not None:
                # s = sum(sign(g - t)); count>=KW  <=>  s >= 2*KW - H
                nc.vector.scalar_tensor_tensor(tneg, lo, -1.0, w,
                                               op0=OP.mult,
                                               op1=OP.subtract)
                nc.scalar.activation(act_scr[:], ytile[:], AF.Sign,
                                     bias=tneg, accum_out=cnt)
                nc.vector.tensor_scalar(pred, cnt, float(2 * KW - H),
                                        None, op0=OP.is_ge)
            else:
                nc.vector.tensor_add(tt_, lo, w)
                nc.vector.scalar_tensor_tensor(
                    get_scr()[:], ytile[:], tt_, ytile[:],
                    op0=OP.is_ge, op1=OP.bypass, accum_out=cnt)
                nc.vector.tensor_scalar(pred, cnt, float(KW), None,
                                        op0=OP.is_ge)
            nc.vector.scalar_tensor_tensor(lo, pred, w, lo,
                                           op0=OP.mult, op1=OP.add)

        def bisect_fin(ytile, bt):
            lo = tiny[bt][:, 1:2]
            nc.vector.scalar_tensor_tensor(ytile[:], ytile[:], lo, ytile[:],
                                           op0=OP.is_ge, op1=OP.mult)

        def bisect_chain(ytile, bt, act_scr=None):
            """One b-tile's top-k bisection + in-place mask.
            act_scr: if given, counts run on ACT via Sign+accum."""
            bisect_init(bt)
            for it in range(NITER):
                bisect_iter(ytile, bt, act_scr)
            bisect_fin(ytile, bt)

        def transpose_bt(ytile, bt, dst_h, dst_l):
            """PE-transpose fp32 y tile; split hi/lo fp16 on the way out."""
            for kb in range(KT_H):
                pt = pst.tile([128, 128], F32, tag="pst", name="ptb")
                nc.tensor.transpose(pt[:],
                                    ytile[:, kb * 128:(kb + 1) * 128],
                                    identity[:])
                cs = slice(kb * 512 + bt * 128, kb * 512 + (bt + 1) * 128)
                nc.scalar.activation(dst_h[:, cs], pt[:], AF.Copy)
                nc.vector.tensor_sub(dst_l[:, cs], pt[:], dst_h[:, cs])

        def transpose_to(ytiles, dst_h, dst_l):
            for bt in range(BT):
                transpose_bt(ytiles[bt], bt, dst_h, dst_l)

        # ---------------- layer 1 ----------------
        sel1 = [selpool.tile([128, H], F32, tag="sel", name=f"sel1_{i}")
                for i in range(BT)]
        mn1 = [mnpool.tile([128, H], F32, tag="mn", name=f"mn1_{i}")
               for i in range(BT)]
        y1 = yphase(xT_sb_h, xT_sb_l, 0, KT_IN, w1t_h, w1t_l, b1sb, H)
        dendphase(sg1a_h, sg1a_l, sg1b_h, sg1b_l, sel1, mn1)
        selgate(sel1, mn1)
        gate_mul(y1, sel1)
        for bt in range(BT):
            bisect_init(bt)

        # layer-1 bisection emitted interleaved with dend2's chunks so the
        # FIFO DVE queue services dend2's PSUM reduces promptly (otherwise
        # the 2 psd banks fill and the PE stalls behind the bisect chains).
        steps = [(bt, j) for j in range(NITER + 1) for bt in range(BT)]
        step_pos = [0]

        def emit_bisect_steps(n):
            for _ in range(n):
                if step_pos[0] >= len(steps):
                    return
                bt, j = steps[step_pos[0]]
                step_pos[0] += 1
                if j < NITER:
                    bisect_iter(y1[bt], bt)
                else:
                    bisect_fin(y1[bt], bt)

        # dend2 early (keeps PE busy during layer-1 bisection)
        sel2 = [selpool.tile([128, H], F32, tag="sel", name=f"sel2_{i}")
                for i in range(BT)]
        mn2 = [mnpool.tile([128, H], F32, tag="mn", name=f"mn2_{i}")
               for i in range(BT)]
        if not INTERLEAVE:
            emit_bisect_steps(len(steps))
        dendphase(sg2a_h, sg2a_l, sg2b_h, sg2b_l, sel2, mn2,
                  post_chunk=(lambda c: emit_bisect_steps(3))
                  if INTERLEAVE else None)
        emit_bisect_steps(len(steps))
        selgate(sel2, mn2)

        transpose_to(y1, h1T_h, h1T_l)

        # ---------------- layer 2 ----------------
        b2sb = load_bias(b2d, H)
        y2 = yphase(h1T_h, h1T_l, 1, KT_H, w2t_h, w2t_l, b2sb, H)
        gate_mul(y2, sel2)

        # per-bt tail pipeline: bisect (DVE/ACT split) -> transpose -> out,
        # so bt0's output matmuls overlap bt1..3's bisection chains.
        bosb = load_bias(bod, D_OUT)
        act_scr = mnpool.tile([128, H], F32, tag="mn", name="act_scr")
        for bt in range(BT):
            bisect_chain(y2[bt], bt, act_scr=(act_scr if bt % 2 else None))
            transpose_bt(y2[bt], bt, h1T_h, h1T_l)
            for n in range(D_OUT // 512):
                ps1 = psy.tile([128, 512], F32, tag="psy", name="pso")
                for k in range(KT_H):
                    wh = wpool.tile([128, 512], F16, tag="w", name="owh")
                    nc.sync.dma_start(wh[:], wot_h[n, k])
                    wl = wpool.tile([128, 512], F16, tag="w", name="owl")
                    nc.sync.dma_start(wl[:], wot_l[n, k])
                    cs = slice(k * 512 + bt * 128, k * 512 + (bt + 1) * 128)
                    nc.tensor.matmul(ps1[:], h1T_h[:, cs], wh[:],
                                     start=(k == 0), stop=False)
                    nc.tensor.matmul(ps1[:], h1T_l[:, cs], wh[:],
                                     start=False, stop=False)
                    nc.tensor.matmul(ps1[:], h1T_h[:, cs], wl[:],
                                     start=False, stop=False)
                nc.tensor.matmul(ps1[:], ones[:],
                                 bosb[:, n * 512:(n + 1) * 512],
                                 start=False, stop=True)
                osb = outpool.tile([128, 512], F32, tag="osb")
                nc.scalar.activation(osb[:], ps1[:], AF.Copy)
                nc.sync.dma_start(
                    out_d[bt * 128:(bt + 1) * 128, n * 512:(n + 1) * 512],
                    osb[:])

    nc.compile()
    return nc


def _split16(a):
    a = np.asarray(a, np.float32)
    hi = a.astype(np.float16)
    lo = (a - hi.astype(np.float32)).astype(np.float16)
    return hi, lo


def _prep_inputs(x, context, w1, b1, seg1, w2, b2, seg2, w_out, b_out):
    """Host-side reshapes into the DMA-friendly tiled layouts + fp16 split."""
    c = np.ascontiguousarray

    def tile_wt(w, kt, nch):
        # w [out, in] -> wT [in, out] -> [nch, kt, 128, 512]
        wT = np.asarray(w, np.float32).T
        return c(wT.reshape(kt, 128, nch, 512).transpose(2, 0, 1, 3))

    def tile_seg(seg):
        segT = np.asarray(seg, np.float32).reshape(HS, D_CTX).T  # [D_CTX, HS]
        a = c(segT[:, :NCH * CHW].reshape(KT_IN, 128, NCH, CHW)
              .transpose(2, 0, 1, 3))
        b = c(segT[:, NCH * CHW:].reshape(KT_IN, 128, TAIL))
        return a, b

    shared = {}
    for name, arr in (("w1t", tile_wt(w1, KT_IN, H // 512)),
                      ("w2t", tile_wt(w2, KT_H, H // 512)),
                      ("wot", tile_wt(w_out, KT_H, D_OUT // 512))):
        hi, lo = _split16(arr)
        shared[name + "_h"] = hi
        shared[name + "_l"] = lo
    for name, seg in (("sg1", seg1), ("sg2", seg2)):
        a, b = tile_seg(seg)
        for suf, arr in (("a", a), ("b", b)):
            hi, lo = _split16(arr)
            shared[name + suf + "_h"] = hi
            shared[name + suf + "_l"] = lo
    shared["b1d"] = c(np.asarray(b1, np.float32).reshape(1, H))
    shared["b2d"] = c(np.asarray(b2, np.float32).reshape(1, H))
    shared["bod"] = c(np.asarray(b_out, np.float32).reshape(1, D_OUT))

    x = np.asarray(x, np.float32)
    context = np.asarray(context, np.float32)
    in_maps = []
    for core in range(NCORES):
        sl = slice(core * BL, (core + 1) * BL)
        m = dict(shared)
        for name, arr in (("xT", c(x[sl].T)), ("ctxT", c(context[sl].T))):
            hi, lo = _split16(arr)
            m[name + "_h"] = hi
            m[name + "_l"] = lo
        in_maps.append(m)
    return in_maps


_NC = None


def kernel(**inputs):
    global _NC
    if _NC is None:
        _NC = build_kernel()
    inputs = {k: np.ascontiguousarray(np.asarray(v), dtype=np.float32)
              for k, v in inputs.items()}
    in_maps = _prep_inputs(**inputs)
    res = run_bass_kernel_spmd(_NC, in_maps, list(range(NCORES)))
    return np.concatenate([res.results[i]["out"] for i in range(NCORES)],
                          axis=0)
